# revision 6
# baseline (speedup 1.0000x reference)
"""Anisotropic Chebyshev graph convolution on 8 Trainium2 NeuronCores.

  out[b,u,m,n] = sum_{k,l,i,p,q} coefs[k,l,i,u] cheb1[k,p,m] cheb2[l,q,n] x[b,i,p,q]

Sharding: data-parallel over batch B=8, one sample per core; cheb1/cheb2/coefs
replicated. Per core, three matmul stages:

  1) a[k,i,m,q]   = sum_p cheb1[k,p,m] x[i,p,q]          (contract p)
  2) c[l,u,m,q]   = sum_{k,i} W[ki,lu] a[k,i,m,q]        (contract k*C)
  3) out[u,m,n]   = sum_{l,q} c[l,u,m,q] cheb2[l,q,n]    (contract q)

Stage 2 needs the (k,i) axis on SBUF partitions while stage 1 naturally
produces m on partitions, so stage 1 writes `a` to DRAM in [m, ki, q] layout
and stage 2 reloads it as [ki, (m,q)] tiles (one layout bounce). Stage 2 uses
the a-tiles as the stationary operand so its output lands with q on
partitions ([q-block, l*U+u]), which stage 3 consumes directly: c-slices
stationary (4 m's packed into the PE array via tile_position col groups),
cheb2 streaming, psum output [(m-phase, u), n].
"""

import numpy as np

import concourse.bacc as bacc
import concourse.bass as bass
import concourse.mybir as mybir
import concourse.tile as tile
from concourse import bass_utils

# Problem shape (hardcoded per contract; matches reference.setup_inputs()).
B = 8
C = 32          # input channels i
U = 32          # output units u
K = 5           # chebyshev powers (k and l)
N1 = 256        # first graph axis (p -> m)
N2 = 256        # second graph axis (q -> n)
P = 128

KI = K * C              # 160, mix contraction size
LU = K * U              # 160, mix output size
LU_PAD = 256            # pad mix output to 256 so fp32r streams at 1 cyc/row
N_CORES = 8

F32 = mybir.dt.float32
F32R = mybir.dt.float32r
BF16 = mybir.dt.bfloat16

# Matmul operand dtype: float32r streams 1 row/cycle (vs 4 for float32) at the
# cost of reduced multiply precision. Flip to F32 if accuracy demands it.
MM_DT = F32R


def _r(ap):
    """Data-path tiles already carry MM_DT; nothing to do."""
    return ap


def build():
    """Build the single-core program (SPMD: every core runs this)."""
    nc = bacc.Bacc("TRN2", target_bir_lowering=False, debug=False, num_devices=1)

    x_d = nc.dram_tensor("x", [C, N1, N2], MM_DT, kind="ExternalInput")
    ch1_d = nc.dram_tensor("cheb1", [K, N1, N1], MM_DT, kind="ExternalInput")
    ch2_d = nc.dram_tensor("cheb2", [K, N2, N2], MM_DT, kind="ExternalInput")
    w1_d = nc.dram_tensor("w1", [P, LU_PAD], MM_DT, kind="ExternalInput")
    w2_d = nc.dram_tensor("w2", [KI - P, LU_PAD], MM_DT, kind="ExternalInput")
    out_d = nc.dram_tensor("out", [U, N1, N2], F32, kind="ExternalOutput")

    NQ = C * N2             # 8192, stage-1 streaming free dim (i,q)
    CHUNK = 512             # stage-1 psum chunk (one bank of fp32)
    NCHUNK = NQ // CHUNK    # 16
    CG = 4                  # psum chunks held across the p-half accumulation
    MG = 8                  # stage-2/3 m-group (reload granularity)

    with tile.TileContext(nc) as tc:
        with (
            tc.tile_pool(name="const", bufs=1) as const_pool,
            tc.tile_pool(name="adram", bufs=1, space="DRAM") as dram_pool,
        ):
            # ---- static inputs -------------------------------------------
            # x as [p, (ph), i*q], one tile per p-half so ph0 matmuls can
            # start while ph1 still loads.
            xp = [
                const_pool.tile([P, NQ], MM_DT, tag=f"xp{ph}", name=f"xp{ph}")
                for ph in range(2)
            ]
            x_r = x_d.ap().rearrange("i (ph p) q -> ph p i q", ph=2)
            for ph in range(2):
                nc.sync.dma_start(
                    xp[ph][:].rearrange("p (i q) -> p i q", q=N2), x_r[ph]
                )

            # cheb1 as [p, k, ph, m]
            ch1 = const_pool.tile([P, K, 2, N1], MM_DT, tag="ch1")
            nc.sync.dma_start(
                ch1[:], ch1_d.ap().rearrange("k (ph p) m -> p k ph m", ph=2)
            )
            # cheb2 as [q, l, qh, n] in bf16: stage 3 runs in bf16 because
            # fp32r matmuls cannot target psum partition offsets != 0 (walrus
            # s3d3_mm_valid_dst_partition), which the col-packed stage-3 needs.
            ch2 = const_pool.tile([P, K, 2, N2], BF16, tag="ch2")
            nc.gpsimd.dma_start(
                ch2[:], ch2_d.ap().rearrange("l (qh q) n -> q l qh n", qh=2)
            )
            # mix weights
            w1 = const_pool.tile([P, LU_PAD], MM_DT, tag="w1")
            nc.sync.dma_start(w1[:], w1_d.ap())
            w2 = const_pool.tile([KI - P, LU_PAD], MM_DT, tag="w2")
            nc.sync.dma_start(w2[:], w2_d.ap())

            # `a` bounce buffer in DRAM, one tile per m-half: [m, ki, q]
            a_dram = [
                dram_pool.tile([P, KI, N2], MM_DT, tag=f"a{mh}", name=f"a{mh}")
                for mh in range(2)
            ]

            # ---- stage 1: a[m, ki, q] = cheb1^T x ------------------------
            evac_flip = 0
            with (
                tc.tile_pool(name="ps_a", bufs=CG + 2, space="PSUM") as ps_a,
                tc.tile_pool(name="ae", bufs=4) as ae_pool,
            ):
                for mh in range(2):
                    for k in range(K):
                        for cg in range(NCHUNK // CG):
                            ps = [
                                ps_a.tile([P, CHUNK], F32, tag="ps_a", name="ps_a")
                                for _ in range(CG)
                            ]
                            for ph in range(2):
                                lhsT = _r(ch1[:, k, ph, mh * P : (mh + 1) * P])
                                for cj in range(CG):
                                    ci = cg * CG + cj
                                    nc.tensor.matmul(
                                        ps[cj][:],
                                        lhsT,
                                        _r(xp[ph][:, ci * CHUNK : (ci + 1) * CHUNK]),
                                        start=(ph == 0),
                                        stop=(ph == 1),
                                    )
                            for cj in range(CG):
                                ci = cg * CG + cj
                                ae = ae_pool.tile([P, 2, N2], MM_DT, tag="ae")
                                if evac_flip == 0:
                                    nc.vector.tensor_copy(ae[:], ps[cj][:])
                                else:
                                    nc.scalar.copy(ae[:], ps[cj][:])
                                evac_flip ^= 1
                                nc.sync.dma_start(
                                    a_dram[mh][
                                        :, k * C + ci * 2 : k * C + ci * 2 + 2, :
                                    ],
                                    ae[:],
                                )

            # ---- stages 2+3 per m-group ----------------------------------
            with (
                tc.tile_pool(name="ps_c", bufs=4, space="PSUM") as ps_c,
                tc.tile_pool(name="ps_o", bufs=3, space="PSUM") as ps_o,
                tc.tile_pool(name="a2", bufs=2) as a2_pool,
                tc.tile_pool(name="csb", bufs=16) as c_pool,
                tc.tile_pool(name="osb", bufs=3) as o_pool,
            ):
                w1r, w2r = _r(w1[:]), _r(w2[:])
                for g in range(N1 // MG):
                    mh, m0 = (g * MG) // P, (g * MG) % P
                    a2a = a2_pool.tile([P, MG, N2], MM_DT, tag="a2a")
                    nc.sync.dma_start(
                        a2a[:],
                        a_dram[mh][m0 : m0 + MG, :P, :].rearrange("m ki q -> ki m q"),
                    )
                    a2b = a2_pool.tile([KI - P, MG, N2], MM_DT, tag="a2b")
                    nc.sync.dma_start(
                        a2b[:],
                        a_dram[mh][m0 : m0 + MG, P:KI, :].rearrange("m ki q -> ki m q"),
                    )
                    for sub in range(MG // 4):
                        # stage 2: c[(q), lu] for the 4 m's and 2 q-halves
                        c_tiles = {}
                        for j in range(4):
                            m_loc = sub * 4 + j
                            for qh in range(2):
                                cps = ps_c.tile([P, LU_PAD], F32, tag="ps_c")
                                nc.tensor.matmul(
                                    cps[:],
                                    _r(a2a[:, m_loc, qh * P : (qh + 1) * P]),
                                    w1r,
                                    start=True,
                                    stop=False,
                                )
                                nc.tensor.matmul(
                                    cps[:],
                                    _r(a2b[:, m_loc, qh * P : (qh + 1) * P]),
                                    w2r,
                                    start=False,
                                    stop=True,
                                )
                                csb = c_pool.tile([P, LU_PAD], BF16, tag="csb")
                                if evac_flip == 0:
                                    nc.vector.tensor_copy(csb[:], cps[:])
                                else:
                                    nc.scalar.copy(csb[:], cps[:])
                                evac_flip ^= 1
                                c_tiles[(j, qh)] = csb

                        # stage 3: out[(m-phase, u), n] += c^T cheb2
                        ops = ps_o.tile([P, N2], F32, tag="ps_o")
                        for l in range(K):
                            for qh in range(2):
                                for j in range(4):
                                    nc.tensor.matmul(
                                        ops[32 * j : 32 * (j + 1), :],
                                        _r(
                                            c_tiles[(j, qh)][
                                                :, l * U : (l + 1) * U
                                            ]
                                        ),
                                        _r(ch2[:, l, qh, :]),
                                        start=(l == 0 and qh == 0),
                                        stop=(l == K - 1 and qh == 1),
                                        tile_position=(0, 32 * j),
                                    )
                        osb = o_pool.tile([P, N2], F32, tag="osb")
                        if evac_flip == 0:
                            nc.vector.tensor_copy(osb[:], ops[:])
                        else:
                            nc.scalar.copy(osb[:], ops[:])
                        evac_flip ^= 1
                        m_abs = g * MG + sub * 4
                        for j in range(4):
                            nc.sync.dma_start(
                                out_d.ap()[:, m_abs + j, :],
                                osb[32 * j : 32 * (j + 1), :],
                            )

    nc.compile()
    return nc


_NC = None
LAST_RUN = {}


def kernel(x, cheb1, cheb2, coefs):
    global _NC
    import time as _time

    if _NC is None:
        t0 = _time.monotonic()
        _NC = build()
        LAST_RUN["build_s"] = _time.monotonic() - t0

    x = np.ascontiguousarray(np.asarray(x, dtype=np.float32))
    cheb1 = np.ascontiguousarray(np.asarray(cheb1, dtype=np.float32))
    cheb2 = np.ascontiguousarray(np.asarray(cheb2, dtype=np.float32))
    coefs = np.asarray(coefs, dtype=np.float32)

    # W[k*C + i, l*U + u] = coefs[k, l, i, u], padded to LU_PAD columns.
    w = coefs.transpose(0, 2, 1, 3).reshape(KI, LU)
    w_pad = np.zeros((KI, LU_PAD), dtype=np.float32)
    w_pad[:, :LU] = w

    in_maps = [
        {
            "x": x[b],
            "cheb1": cheb1,
            "cheb2": cheb2,
            "w1": np.ascontiguousarray(w_pad[:P]),
            "w2": np.ascontiguousarray(w_pad[P:]),
        }
        for b in range(B)
    ]

    t0 = _time.monotonic()
    res = bass_utils.run_bass_kernel_spmd(_NC, in_maps, core_ids=list(range(N_CORES)))
    LAST_RUN["wall_s"] = _time.monotonic() - t0
    LAST_RUN["exec_time_ns"] = res.exec_time_ns

    return np.stack([res.results[b]["out"] for b in range(B)])


# revision 11
# speedup vs baseline: 15314.3610x; 15314.3610x over previous
"""Anisotropic Chebyshev graph convolution on 8 Trainium2 NeuronCores.

  out[b,u,m,n] = sum_{k,l,i,p,q} coefs[k,l,i,u] cheb1[k,p,m] cheb2[l,q,n] x[b,i,p,q]

Sharding: data-parallel over batch B=8, one sample per core; cheb1/cheb2/coefs
replicated. Per core, three matmul stages:

  1) a[k,i,m,q]   = sum_p cheb1[k,p,m] x[i,p,q]          (contract p)
  2) c[l,u,m,q]   = sum_{k,i} W[ki,lu] a[k,i,m,q]        (contract k*C)
  3) out[u,m,n]   = sum_{l,q} c[l,u,m,q] cheb2[l,q,n]    (contract q)

Stage 2 needs the (k,i) axis on SBUF partitions while stage 1 naturally
produces m on partitions, so stage 1 writes `a` to DRAM in [m, ki, q] layout
and stage 2 reloads it as [ki, (m,q)] tiles (one layout bounce). Stage 2 uses
the a-tiles as the stationary operand so its output lands with q on
partitions ([q-block, l*U+u]), which stage 3 consumes directly: c-slices
stationary (4 m's packed into the PE array via tile_position col groups),
cheb2 streaming, psum output [(m-phase, u), n].
"""

import numpy as np

import concourse.bacc as bacc
import concourse.bass as bass
import concourse.mybir as mybir
import concourse.tile as tile
from concourse import bass_utils

# Problem shape (hardcoded per contract; matches reference.setup_inputs()).
B = 8
C = 32          # input channels i
U = 32          # output units u
K = 5           # chebyshev powers (k and l)
N1 = 256        # first graph axis (p -> m)
N2 = 256        # second graph axis (q -> n)
P = 128

KI = K * C              # 160, mix contraction size
LU = K * U              # 160, mix output size
LU_PAD = 256            # pad mix output to 256 so fp32r streams at 1 cyc/row
N_CORES = 8

F32 = mybir.dt.float32
F32R = mybir.dt.float32r
BF16 = mybir.dt.bfloat16

# Matmul operand dtype: float32r streams 1 row/cycle (vs 4 for float32) at the
# cost of reduced multiply precision. Flip to F32 if accuracy demands it.
MM_DT = F32R


def _r(ap):
    """Data-path tiles already carry MM_DT; nothing to do."""
    return ap


def build(n_iters=1):
    """Build the single-core program (SPMD: every core runs this).

    n_iters > 1 repeats the full pipeline (loads included) for wall-clock
    delta timing through the high fixed axon dispatch overhead.
    """
    nc = bacc.Bacc("TRN2", target_bir_lowering=False, debug=False, num_devices=1)

    x_d = nc.dram_tensor("x", [C, N1, N2], MM_DT, kind="ExternalInput")
    ch1_d = nc.dram_tensor("cheb1", [K, N1, N1], MM_DT, kind="ExternalInput")
    ch2_d = nc.dram_tensor("cheb2", [K, N2, N2], MM_DT, kind="ExternalInput")
    w1_d = nc.dram_tensor("w1", [P, LU_PAD], MM_DT, kind="ExternalInput")
    w2_d = nc.dram_tensor("w2", [KI - P, LU_PAD], MM_DT, kind="ExternalInput")
    out_d = nc.dram_tensor("out", [U, N1, N2], F32, kind="ExternalOutput")

    NQ = C * N2             # 8192, stage-1 streaming free dim (i,q)
    CHUNK = 512             # stage-1 psum chunk (one bank of fp32)
    NCHUNK = NQ // CHUNK    # 16
    CG = 4                  # psum chunks held across the p-half accumulation
    MG = 8                  # stage-2/3 m-group (reload granularity)

    with tile.TileContext(nc) as tc:
      for _it in range(n_iters):
        with (
            tc.tile_pool(name="const", bufs=1) as const_pool,
            tc.tile_pool(name="adram", bufs=1, space="DRAM") as dram_pool,
        ):
            # ---- static inputs -------------------------------------------
            # x as [p, (ph), i*q], one tile per p-half so ph0 matmuls can
            # start while ph1 still loads.
            xp = [
                const_pool.tile([P, NQ], MM_DT, tag=f"xp{ph}", name=f"xp{ph}")
                for ph in range(2)
            ]
            x_r = x_d.ap().rearrange("i (ph p) q -> ph p i q", ph=2)
            for ph in range(2):
                nc.sync.dma_start(
                    xp[ph][:].rearrange("p (i q) -> p i q", q=N2), x_r[ph]
                )

            # cheb1 as [p, k, ph, m]
            ch1 = const_pool.tile([P, K, 2, N1], MM_DT, tag="ch1")
            nc.sync.dma_start(
                ch1[:], ch1_d.ap().rearrange("k (ph p) m -> p k ph m", ph=2)
            )
            # cheb2 as [q, l, qh, n]
            ch2 = const_pool.tile([P, K, 2, N2], MM_DT, tag="ch2")
            nc.sync.dma_start(
                ch2[:], ch2_d.ap().rearrange("l (qh q) n -> q l qh n", qh=2)
            )
            # mix weights
            w1 = const_pool.tile([P, LU_PAD], MM_DT, tag="w1")
            nc.sync.dma_start(w1[:], w1_d.ap())
            w2 = const_pool.tile([KI - P, LU_PAD], MM_DT, tag="w2")
            nc.sync.dma_start(w2[:], w2_d.ap())

            # `a` bounce buffer in DRAM, one tile per m-half: [m, ki, q]
            a_dram = [
                dram_pool.tile([P, KI, N2], MM_DT, tag=f"a{mh}", name=f"a{mh}")
                for mh in range(2)
            ]

            # ---- stage 1: a[m, ki, q] = cheb1^T x ------------------------
            evac_flip = 0
            with (
                tc.tile_pool(name="ps_a", bufs=CG + 2, space="PSUM") as ps_a,
                tc.tile_pool(name="ae", bufs=4) as ae_pool,
            ):
                for mh in range(2):
                    for k in range(K):
                        for cg in range(NCHUNK // CG):
                            ps = [
                                ps_a.tile([P, CHUNK], F32, tag="ps_a", name="ps_a")
                                for _ in range(CG)
                            ]
                            for ph in range(2):
                                lhsT = _r(ch1[:, k, ph, mh * P : (mh + 1) * P])
                                for cj in range(CG):
                                    ci = cg * CG + cj
                                    nc.tensor.matmul(
                                        ps[cj][:],
                                        lhsT,
                                        _r(xp[ph][:, ci * CHUNK : (ci + 1) * CHUNK]),
                                        start=(ph == 0),
                                        stop=(ph == 1),
                                    )
                            for cj in range(CG):
                                ci = cg * CG + cj
                                ae = ae_pool.tile([P, 2, N2], MM_DT, tag="ae")
                                if evac_flip == 0:
                                    nc.vector.tensor_copy(ae[:], ps[cj][:])
                                else:
                                    nc.scalar.copy(ae[:], ps[cj][:])
                                evac_flip ^= 1
                                nc.sync.dma_start(
                                    a_dram[mh][
                                        :, k * C + ci * 2 : k * C + ci * 2 + 2, :
                                    ],
                                    ae[:],
                                )

            # ---- stages 2+3 per m-group ----------------------------------
            with (
                tc.tile_pool(name="ps_c", bufs=4, space="PSUM") as ps_c,
                tc.tile_pool(name="ps_o", bufs=3, space="PSUM") as ps_o,
                tc.tile_pool(name="a2", bufs=2) as a2_pool,
                tc.tile_pool(name="csb", bufs=4) as c_pool,
                tc.tile_pool(name="osb", bufs=3) as o_pool,
            ):
                w1r, w2r = _r(w1[:]), _r(w2[:])
                for g in range(N1 // MG):
                    mh, m0 = (g * MG) // P, (g * MG) % P
                    a2a = a2_pool.tile([P, MG, N2], MM_DT, tag="a2a")
                    nc.sync.dma_start(
                        a2a[:],
                        a_dram[mh][m0 : m0 + MG, :P, :].rearrange("m ki q -> ki m q"),
                    )
                    a2b = a2_pool.tile([KI - P, MG, N2], MM_DT, tag="a2b")
                    nc.sync.dma_start(
                        a2b[:],
                        a_dram[mh][m0 : m0 + MG, P:KI, :].rearrange("m ki q -> ki m q"),
                    )
                    for sub in range(MG // 4):
                        # stage 2: c[(q), mj, lu] for the 4 m's and 2 q-halves,
                        # evacuated into an (mj, lu)-interleaved tile so stage 3
                        # can consume 128 output partitions (4 m's x 32 u) with
                        # ONE matmul per (l, qh) instead of 4 col-packed ones.
                        # csb layout [q, l, mj, u]: each l-slice is one
                        # contiguous 128-wide block (4 m's x 32 u) so the
                        # stage-3 stationary AP has a single free dimension.
                        c_tiles = []
                        for qh in range(2):
                            csb = c_pool.tile([P, K, 4, U], MM_DT, tag="csb")
                            c_tiles.append(csb)
                            for j in range(4):
                                m_loc = sub * 4 + j
                                cps = ps_c.tile([P, LU_PAD], F32, tag="ps_c")
                                nc.tensor.matmul(
                                    cps[:],
                                    _r(a2a[:, m_loc, qh * P : (qh + 1) * P]),
                                    w1r,
                                    start=True,
                                    stop=False,
                                )
                                nc.tensor.matmul(
                                    cps[:],
                                    _r(a2b[:, m_loc, qh * P : (qh + 1) * P]),
                                    w2r,
                                    start=False,
                                    stop=True,
                                )
                                src = cps[:, :LU].rearrange("p (l u) -> p l u", u=U)
                                if evac_flip == 0:
                                    nc.vector.tensor_copy(csb[:, :, j, :], src)
                                else:
                                    nc.scalar.copy(csb[:, :, j, :], src)
                                evac_flip ^= 1

                        # stage 3: out[(mj, u), n] += c^T cheb2, full-width
                        ops = ps_o.tile([P, N2], F32, tag="ps_o")
                        for l in range(K):
                            for qh in range(2):
                                nc.tensor.matmul(
                                    ops[:],
                                    _r(c_tiles[qh][:, l]),
                                    _r(ch2[:, l, qh, :]),
                                    start=(l == 0 and qh == 0),
                                    stop=(l == K - 1 and qh == 1),
                                )
                        osb = o_pool.tile([P, N2], F32, tag="osb")
                        if evac_flip == 0:
                            nc.vector.tensor_copy(osb[:], ops[:])
                        else:
                            nc.scalar.copy(osb[:], ops[:])
                        evac_flip ^= 1
                        m_abs = g * MG + sub * 4
                        for j in range(4):
                            nc.sync.dma_start(
                                out_d.ap()[:, m_abs + j, :],
                                osb[32 * j : 32 * (j + 1), :],
                            )

    nc.compile()
    return nc


_NC = None
LAST_RUN = {}


def kernel(x, cheb1, cheb2, coefs):
    global _NC
    import time as _time

    if _NC is None:
        t0 = _time.monotonic()
        _NC = build()
        LAST_RUN["build_s"] = _time.monotonic() - t0

    x = np.ascontiguousarray(np.asarray(x, dtype=np.float32))
    cheb1 = np.ascontiguousarray(np.asarray(cheb1, dtype=np.float32))
    cheb2 = np.ascontiguousarray(np.asarray(cheb2, dtype=np.float32))
    coefs = np.asarray(coefs, dtype=np.float32)

    # W[k*C + i, l*U + u] = coefs[k, l, i, u], padded to LU_PAD columns.
    w = coefs.transpose(0, 2, 1, 3).reshape(KI, LU)
    w_pad = np.zeros((KI, LU_PAD), dtype=np.float32)
    w_pad[:, :LU] = w

    in_maps = [
        {
            "x": x[b],
            "cheb1": cheb1,
            "cheb2": cheb2,
            "w1": np.ascontiguousarray(w_pad[:P]),
            "w2": np.ascontiguousarray(w_pad[P:]),
        }
        for b in range(B)
    ]

    t0 = _time.monotonic()
    res = bass_utils.run_bass_kernel_spmd(_NC, in_maps, core_ids=list(range(N_CORES)))
    LAST_RUN["wall_s"] = _time.monotonic() - t0
    LAST_RUN["exec_time_ns"] = res.exec_time_ns

    return np.stack([res.results[b]["out"] for b in range(B)])
